# revision 1
# baseline (speedup 1.0000x reference)
"""Trainium2 Bass kernel for CustomConv2d:
  x [16, 32, 512, 512] f32, weight [32, 32, 3, 3] f32, bias [32] f32
  -> out [16, 32, 510, 510] f32   (stride 1, VALID padding, + bias)

Data-parallel over batch: 2 images per core across 8 NeuronCores.

v2 design (pair layouts for 4KB HBM DMA descriptors):
 - x SBUF layout: partition 32g+ci holds image row r (g=(r//2)%4), free
   offset 1024*(r//8 - slot0) + 512*(r%2). Input DMA descriptors cover 2
   consecutive rows (4KB contiguous HBM).
 - block = 8 output rows (y0 % 8 == 0): row p -> PSUM partitions 32*(p//2),
   sub-slot p%2. Output partition 32j+co holds rows y0+2j, y0+2j+1 at
   consecutive free offsets -> 4080B-contiguous output descriptors.
 - conv: per row 2-3 bf16 K<=64 matmuls on the 64x32-tiled PE (tile A =
   partitions 0-63, B = 64-127), grouped by (tile, row parity, slot).
   Banks are tile-pure: (sub, A), (sub, B); empty (bank, col) groups get a
   single zero-weight matmul so drains stay uniform [128-partition].
 - drain: ScalarE Identity(bankB + bias) -> t; VectorE t + bankA -> ostrip.
 - weights: one contiguous DMA + DVE 32x32 transposes + 48 small SBUF->SBUF
   block copies build 22 distinct [64, 32] variants; cast once to bf16.
"""
import numpy as np

import concourse.bass as bass
import concourse.tile as tile
from concourse import bacc, mybir
from concourse.bass_utils import run_bass_kernel_spmd
from contextlib import ExitStack

F32 = mybir.dt.float32
BF16 = mybir.dt.bfloat16

N_FULL, C, H, W = 16, 32, 512, 512
HO = WO = 510
N_CORES = 8
N_PER = N_FULL // N_CORES
N_STRIPS = H // 32                 # 16 strips of 32 rows
XCOLS = 4 * 1024                   # strip tile free size (4 slots-of-8)


def _mm_table():
    """Per phase p (row offset in block, 0..7): list of MMs
    (T, o, sl, items) with items = [(gi, kh)], grouped by (T, o, sl)."""
    table = []
    for p in range(8):
        groups = {}
        for kh in range(3):
            r = p + kh
            g = (r // 2) % 4
            T = 0 if g < 2 else 1
            groups.setdefault((T, r % 2, r // 8), []).append((g % 2, kh))
        table.append([(T, o, sl, tuple(items))
                      for (T, o, sl), items in groups.items()])
    return table

MM_TABLE = _mm_table()

# distinct variant contents: key = tuple(sorted(items)); plus the zero key ()
_KEYS = sorted({tuple(sorted(it)) for row in MM_TABLE for (_, _, _, it) in row})
# column layout: for kw in 0..2: one col per key; then one shared zero col
WCOL = {}
for k in _KEYS:
    for kw in range(3):
        WCOL[(k, kw)] = len(WCOL)
ZCOL = len(WCOL)
NWCOL = ZCOL + 1                    # 22 columns of 32


def _build():
    nc = bacc.Bacc("TRN2", target_bir_lowering=False, debug=False, num_devices=1)
    x_d = nc.dram_tensor("x", [N_PER, C, H, W], F32, kind="ExternalInput").ap()
    w_d = nc.dram_tensor("w", [C, C, 3, 3], F32, kind="ExternalInput").ap()
    b_d = nc.dram_tensor("b", [C], F32, kind="ExternalInput").ap()
    o_d = nc.dram_tensor("out", [N_PER, C, HO, WO], F32, kind="ExternalOutput").ap()

    with tile.TileContext(nc) as tc, ExitStack() as ctx:
        const_pool = ctx.enter_context(tc.tile_pool(name="const", bufs=1))
        xf_pool = ctx.enter_context(tc.tile_pool(name="xf", bufs=2))
        xb_pool = ctx.enter_context(tc.tile_pool(name="xb", bufs=3))
        psum_pool = ctx.enter_context(tc.tile_pool(name="ps", bufs=2, space="PSUM"))
        t_pool = ctx.enter_context(tc.tile_pool(name="t", bufs=4))
        out_pool = ctx.enter_context(tc.tile_pool(name="ostrip", bufs=3))

        # ---- weights: contiguous load + on-chip transpose + block copies ----
        # wstage[co, ci*9 + kh*3 + kw]
        wstage = const_pool.tile([32, 288], F32)
        nc.sync.dma_start(wstage[:], w_d[:].rearrange("o i h w -> o (i h w)"))
        # wT[ci, 32*(kh*3+kw) + co] via 9 DVE 32x32 block transposes
        wg = const_pool.tile([32, 288], F32)
        for t9 in range(9):
            nc.vector.tensor_copy(wg[:, 32 * t9:32 * t9 + 32],
                                  wstage[:, t9:288:9])
        wT = const_pool.tile([32, 288], F32)
        nc.vector.transpose(wT[:], wg[:])
        # variant image [128, NWCOL*32] fp32: A copy at partitions 0-63,
        # B copy at 64-127 (same contents)
        wf = const_pool.tile([128, NWCOL * 32], F32)
        nc.vector.memset(wf[:], 0.0)
        for key in _KEYS:
            c0 = WCOL[(key, 0)] * 32          # 3 kw cols are adjacent
            for gi, kh in key:
                for base in (0, 64):
                    nc.sync.dma_start(
                        wf[base + 32 * gi:base + 32 * gi + 32, c0:c0 + 96],
                        wT[:, 96 * kh:96 * kh + 96])
        wb = const_pool.tile([128, NWCOL * 32], BF16)
        nc.vector.tensor_copy(wb[:], wf[:])
        bt = const_pool.tile([128, 1], F32)
        for j in range(4):
            nc.gpsimd.dma_start(bt[32 * j:32 * j + 32, 0:1], b_d[:].unsqueeze(1))

        # queue discipline: input slot DMAs alternate the two HWDGE queues
        # (sync/scalar); output block DMAs ride the async SWDGE (gpsimd)
        # so they never head-of-line-block the ACT drain stream.
        uid = [0]
        in_rr = [0]

        def in_dma(dst, src):
            eng = nc.scalar if in_rr[0] % 2 == 0 else nc.gpsimd
            in_rr[0] += 1
            eng.dma_start(dst, src)

        def out_dma(dst, src):
            nc.sync.dma_start(dst, src)

        def emit_block(xb_cur, xb_next, b8, ostrip, nrow=8):
            """MMs + drain for one block (nrow output rows, y0 = 32s+8*b8).
            Block uses slots b8, b8+1 of the strip tile; slot 4 -> xb_next
            slot 0."""
            psA = {}
            psB = {}
            for sub in range(2):
                uid[0] += 1
                psA[sub] = psum_pool.tile([128, 512], F32, tag=f"psA{sub}",
                                          name=f"psA{sub}_{uid[0]}")
                psB[sub] = psum_pool.tile([128, 512], F32, tag=f"psB{sub}",
                                          name=f"psB{sub}_{uid[0]}")
            banks = {0: psA, 1: psB}
            njs = (nrow + 1) // 2
            # collect MM lists per (T, sub, j)
            groups = {}
            for kw in range(3):
                for p in range(nrow):
                    j, sub = p // 2, p % 2
                    for (T, o, sl, items) in MM_TABLE[p]:
                        col = WCOL[(tuple(sorted(items)), kw)]
                        groups.setdefault((T, sub, j), []).append(
                            (o, sl, col, kw))
            # zero-pad empty (T, sub, j) combos so drains are uniform
            for sub in range(2):
                for j in range(njs):
                    for T in range(2):
                        if (T, sub, j) not in groups:
                            groups[(T, sub, j)] = [(0, 0, ZCOL, 0)]
            # emit: kw-major over the collected lists, preserving per-group
            # order for start/stop flags
            idx = {k: 0 for k in groups}
            order = []
            for k, mms in groups.items():
                for i, m in enumerate(mms):
                    order.append((m[3], k, i, m))
            order.sort(key=lambda e: (e[0], e[2], e[1][2], e[1][1], e[1][0]))
            for _, (T, sub, j), i, (o, sl, col, kw) in order:
                ps = banks[T][sub]
                xa = xb_cur
                if sl + b8 >= 4:
                    xa = xb_next
                soff = (sl + b8) % 4 * 1024 + 512 * o + kw
                p0 = 0 if T == 0 else 64
                n_mms = len(groups[(T, sub, j)])
                nc.tensor.matmul(
                    ps[32 * j:32 * j + 32, 0:WO],
                    wb[p0:p0 + 64, 32 * col:32 * col + 32],
                    xa[p0:p0 + 64, soff:soff + WO],
                    start=(i == 0), stop=(i == n_mms - 1),
                    skip_group_check=True,
                    tile_position=(p0, 32 * j),
                )
            # drains: per sub, ACT Identity(B + bias) then DVE + A
            npart = 32 * njs
            for sub in range(2):
                uid[0] += 1
                t = t_pool.tile([128, WO], F32, tag="t", name=f"t_{uid[0]}")
                nc.scalar.activation(t[0:npart, :], psB[sub][0:npart, 0:WO],
                                     mybir.ActivationFunctionType.Identity,
                                     bias=bt[0:npart, :])
                nc.vector.tensor_add(
                    ostrip[0:npart, b8 * 1020 + 510 * sub:
                           b8 * 1020 + 510 * sub + WO],
                    t[0:npart, :], psA[sub][0:npart, 0:WO])

        def dma_out_strip(n, s, ostrip):
            # pairs of rows per descriptor: rows 32s + 8*b8 + 2j + e
            nb8 = 4 if s < N_STRIPS - 1 else 3
            rows = o_d[n, :, 32 * s:32 * s + 8 * nb8, :].rearrange(
                "c (b8 j2 e) w -> j2 c b8 (e w)", b8=nb8, j2=4, e=2)
            for j in range(4):
                out_dma(rows[j], ostrip[32 * j:32 * j + 32, 0:nb8 * 1020])
            if s == N_STRIPS - 1:
                for j in range(3):
                    dst = o_d[n, :, 504 + 2 * j:506 + 2 * j, :].rearrange(
                        "c e w -> c (e w)")
                    out_dma(dst, ostrip[32 * j:32 * j + 32,
                                        3 * 1020:3 * 1020 + 1020])

        for n in range(N_PER):
            prev = None  # (xb_prev, ostrip_prev, strip_idx)
            for s in range(N_STRIPS):
                uid[0] += 1
                xf = xf_pool.tile([128, XCOLS], F32, tag="xf",
                                  name=f"xf_{uid[0]}")
                xsrc = x_d[n, :, 32 * s:32 * s + 32, :].rearrange(
                    "c (t g2 e) w -> g2 c t (e w)", t=4, g2=4, e=2)
                for g in range(4):
                    in_dma(xf[32 * g:32 * g + 32, :], xsrc[g])
                xb = xb_pool.tile([128, XCOLS], BF16, tag="xb",
                                  name=f"xb_{uid[0]}")
                nc.vector.tensor_copy(xb[:], xf[:])

                if prev is not None:
                    xbp, osp, sp = prev
                    emit_block(xbp, xb, 3, osp)
                    dma_out_strip(n, sp, osp)
                uid[0] += 1
                ostrip = out_pool.tile([128, 4 * 1020], F32, tag="ostrip",
                                       name=f"os_{uid[0]}")
                if s < N_STRIPS - 1:
                    for b8 in range(3):
                        emit_block(xb, None, b8, ostrip)
                    prev = (xb, ostrip, s)
                else:
                    for b8 in range(3):
                        emit_block(xb, None, b8, ostrip)
                    emit_block(xb, None, 3, ostrip, nrow=6)
                    dma_out_strip(n, s, ostrip)
                    prev = None

    nc.compile()
    return nc


_NC = None


def kernel(x, weight, bias):
    global _NC
    x = np.ascontiguousarray(np.asarray(x, dtype=np.float32))
    weight = np.ascontiguousarray(np.asarray(weight, dtype=np.float32))
    bias = np.ascontiguousarray(np.asarray(bias, dtype=np.float32))
    if _NC is None:
        _NC = _build()
    in_maps = [
        {"x": x[N_PER * i:N_PER * (i + 1)], "w": weight, "b": bias}
        for i in range(N_CORES)
    ]
    res = run_bass_kernel_spmd(_NC, in_maps, core_ids=list(range(N_CORES)))
    return np.concatenate([r["out"] for r in res.results], axis=0)



# revision 2
# speedup vs baseline: 1.8570x; 1.8570x over previous
"""Trainium2 Bass kernel for CustomConv2d:
  x [16, 32, 512, 512] f32, weight [32, 32, 3, 3] f32, bias [32] f32
  -> out [16, 32, 510, 510] f32   (stride 1, VALID padding, + bias)

Data-parallel over batch: 2 images per core across 8 NeuronCores.

v3 design — bf16 HBM I/O + host-side layout swizzle (rel-err budget 2e-2,
bf16 compute was already in use; storing bf16 in HBM halves DMA bytes):
 - host pre-swizzles x into the exact SBUF strip layout, bf16:
   xs[img*16+s, 32g+ci, 512k+w] = x[img, ci, 32s+4k+g, w].  Every input
   DMA descriptor is 8KB contiguous; one 1MB dma_start per strip.
 - mod-4 row rotation: partition group g holds rows r = g (mod 4).  An
   output row's 3 kh taps land in 3 cyclically-consecutive groups, so each
   row needs exactly one K=64 matmul on PE row-half T0 (parts 0-63) and one
   on T1 (64-127) per kw: 48 MMs per 8-row block (75% PE efficiency),
   perfectly uniform 3-MM accumulation groups (kw 0..2), no zero padding.
 - weights: host builds the 12 [64,32] bf16 variants (4 tap patterns x 3
   kw) directly; single tiny DMA, no on-chip transposes.
 - drain: ACT Identity(psB + bias) -> t (f32); DVE t + psA -> ostrip bf16.
 - output written bf16 in drain-native layout (8160B descriptors, one 1MB
   dma_start per strip); host de-swizzles and upcasts to f32.
"""
import numpy as np
import ml_dtypes

import concourse.bass as bass
import concourse.tile as tile
from concourse import bacc, mybir
from concourse.bass_utils import run_bass_kernel_spmd
from contextlib import ExitStack

F32 = mybir.dt.float32
BF16 = mybir.dt.bfloat16
BF = ml_dtypes.bfloat16

N_FULL, C, H, W = 16, 32, 512, 512
HO = WO = 510
N_CORES = 8
N_PER = N_FULL // N_CORES
N_STRIPS = H // 32                  # 16 strips of 32 rows
NS = N_PER * N_STRIPS               # strip-tensors per core

# tap patterns (content of the two 32-part groups of one PE row-half):
# entries are kh indices or None (zero block)
PATTERNS = [(0, 1), (None, 0), (2, None), (1, 2)]
# per m = y%4: (T0 pattern idx, T0 slot shift, T1 pattern idx, T1 slot shift)
ROW_TABLE = [(0, 0, 2, 0),
             (1, 0, 3, 0),
             (2, 1, 0, 0),
             (3, 1, 1, 0)]


def _build():
    nc = bacc.Bacc("TRN2", target_bir_lowering=False, debug=False, num_devices=1)
    x_d = nc.dram_tensor("xs", [NS, 128, 4096], BF16, kind="ExternalInput").ap()
    w_d = nc.dram_tensor("wb", [128, 384], BF16, kind="ExternalInput").ap()
    b_d = nc.dram_tensor("bt", [128, 1], F32, kind="ExternalInput").ap()
    o_d = nc.dram_tensor("out", [NS, 128, 4080], BF16, kind="ExternalOutput").ap()

    with tile.TileContext(nc) as tc, ExitStack() as ctx:
        const_pool = ctx.enter_context(tc.tile_pool(name="const", bufs=1))
        xb_pool = ctx.enter_context(tc.tile_pool(name="xb", bufs=4))
        psum_pool = ctx.enter_context(tc.tile_pool(name="ps", bufs=2, space="PSUM"))
        t_pool = ctx.enter_context(tc.tile_pool(name="t", bufs=4))
        out_pool = ctx.enter_context(tc.tile_pool(name="ostrip", bufs=3))

        wb = const_pool.tile([128, 384], BF16)
        nc.sync.dma_start(wb[:], w_d[:])
        bt = const_pool.tile([128, 1], F32)
        nc.scalar.dma_start(bt[:], b_d[:])

        # round-robin DMA queues; reads and writes offset so each queue
        # carries ~1/3 of each direction
        engs = [nc.sync, nc.scalar, nc.gpsimd]
        rr = [0]

        def in_dma(dst, src):
            engs[rr[0] % 3].dma_start(dst, src)
            rr[0] += 1

        def out_dma(dst, src):
            engs[(rr[0] + 1) % 3].dma_start(dst, src)
            rr[0] += 1

        uid = [0]

        def emit_block(xcur, xnext, b8, ostrip, nrow=8):
            """48 MMs + 2 drains for one block (rows y0..y0+nrow-1,
            y0 = 32s + 8*b8)."""
            uid[0] += 1
            psA = {}
            psB = {}
            for sub in range(2):
                psA[sub] = psum_pool.tile([128, 512], F32, tag=f"psA{sub}",
                                          name=f"psA{sub}_{uid[0]}")
                psB[sub] = psum_pool.tile([128, 512], F32, tag=f"psB{sub}",
                                          name=f"psB{sub}_{uid[0]}")
            for kw in range(3):
                for p in range(nrow):
                    j, sub = p // 2, p % 2
                    k = 2 * b8 + p // 4
                    p0v, d0, p1v, d1 = ROW_TABLE[p % 4]
                    for T, pv, dd, ps in ((0, p0v, d0, psA[sub]),
                                          (1, p1v, d1, psB[sub])):
                        sl = k + dd
                        xa = xcur
                        if sl >= 8:
                            xa = xnext
                            sl -= 8
                        off = sl * 512 + kw
                        base = 64 * T
                        v = pv * 3 + kw
                        nc.tensor.matmul(
                            ps[32 * j:32 * j + 32, 0:WO],
                            wb[base:base + 64, 32 * v:32 * v + 32],
                            xa[base:base + 64, off:off + WO],
                            start=(kw == 0), stop=(kw == 2),
                            skip_group_check=True,
                            tile_position=(base, 32 * j),
                        )
            npart = 32 * ((nrow + 1) // 2)
            for sub in range(2):
                uid[0] += 1
                t = t_pool.tile([128, WO], F32, tag="t", name=f"t_{uid[0]}")
                nc.scalar.activation(t[0:npart, :], psB[sub][0:npart, 0:WO],
                                     mybir.ActivationFunctionType.Identity,
                                     bias=bt[0:npart, :])
                nc.vector.tensor_add(
                    ostrip[0:npart, 1020 * b8 + 510 * sub:
                           1020 * b8 + 510 * sub + WO],
                    t[0:npart, :], psA[sub][0:npart, 0:WO])

        for n in range(N_PER):
            prev = None  # (xb_prev, ostrip_prev, strip_idx)
            for s in range(N_STRIPS):
                uid[0] += 1
                xb = xb_pool.tile([128, 4096], BF16, tag="xb",
                                  name=f"xb_{uid[0]}")
                in_dma(xb[:], x_d[n * N_STRIPS + s])

                if prev is not None:
                    xbp, osp, sp = prev
                    emit_block(xbp, xb, 3, osp)
                    out_dma(o_d[n * N_STRIPS + sp], osp[:])
                uid[0] += 1
                ostrip = out_pool.tile([128, 4080], BF16, tag="ostrip",
                                       name=f"os_{uid[0]}")
                for b8 in range(3):
                    emit_block(xb, None, b8, ostrip)
                if s < N_STRIPS - 1:
                    prev = (xb, ostrip, s)
                else:
                    emit_block(xb, None, 3, ostrip, nrow=6)
                    # last block drains only partitions 0..95; avoid DMAing
                    # the never-written corner of the tile
                    idx = n * N_STRIPS + s
                    out_dma(o_d[idx, 0:96], ostrip[0:96, :])
                    out_dma(o_d[idx, 96:128, 0:3060], ostrip[96:128, 0:3060])
                    prev = None

    nc.compile()
    return nc


def _prep_x(x):
    """[16, 32, 512, 512] f32 -> per-core list of [32, 128, 4096] bf16."""
    xb = x.astype(BF)
    cores = []
    for c in range(N_CORES):
        imgs = []
        for n in range(N_PER):
            im = xb[c * N_PER + n]                      # [32, 512, 512]
            im = im.reshape(C, N_STRIPS, 8, 4, W)       # ci, s, k, g, w
            im = im.transpose(1, 3, 0, 2, 4)            # s, g, ci, k, w
            imgs.append(np.ascontiguousarray(im.reshape(N_STRIPS, 128, 4096)))
        cores.append(np.concatenate(imgs, axis=0))
    return cores


def _prep_w(weight):
    """[32, 32, 3, 3] f32 -> [128, 384] bf16 variant image."""
    wb = np.zeros((128, 384), dtype=np.float32)
    for pi, (a, b) in enumerate(PATTERNS):
        for kw in range(3):
            col = 32 * (pi * 3 + kw)
            for T in (0, 64):
                if a is not None:
                    wb[T:T + 32, col:col + 32] = weight[:, :, a, kw].T
                if b is not None:
                    wb[T + 32:T + 64, col:col + 32] = weight[:, :, b, kw].T
    return wb.astype(BF)


def _unprep_out(o_arrs):
    """per-core [32, 128, 4080] bf16 -> [16, 32, 510, 510] f32."""
    full = np.empty((N_FULL, C, HO, WO), dtype=np.float32)
    for c, arr in enumerate(o_arrs):
        a = np.asarray(arr).reshape(N_PER, N_STRIPS, 4, 32, 4, 2, WO)
        # dims: n, s, j, co, b8, e, w  ->  n, co, s, b8, j, e, w
        a = a.transpose(0, 3, 1, 4, 2, 5, 6).reshape(N_PER, C, 512, WO)
        full[c * N_PER:(c + 1) * N_PER] = a[:, :, :HO, :].astype(np.float32)
    return full


_NC = None


def prepare_in_maps(x, weight, bias):
    x = np.ascontiguousarray(np.asarray(x, dtype=np.float32))
    weight = np.ascontiguousarray(np.asarray(weight, dtype=np.float32))
    bias = np.ascontiguousarray(np.asarray(bias, dtype=np.float32))
    xs = _prep_x(x)
    wb = _prep_w(weight)
    bt = np.repeat(bias.reshape(1, 32), 4, axis=0).reshape(128, 1)
    bt = np.ascontiguousarray(bt, dtype=np.float32)
    return [{"xs": xs[i], "wb": wb, "bt": bt} for i in range(N_CORES)]


def kernel(x, weight, bias):
    global _NC
    if _NC is None:
        _NC = _build()
    in_maps = prepare_in_maps(x, weight, bias)
    res = run_bass_kernel_spmd(_NC, in_maps, core_ids=list(range(N_CORES)))
    return _unprep_out([r["out"] for r in res.results])


# revision 6
# speedup vs baseline: 1.8681x; 1.0060x over previous
"""Trainium2 Bass kernel for CustomConv2d:
  x [16, 32, 512, 512] f32, weight [32, 32, 3, 3] f32, bias [32] f32
  -> out [16, 32, 510, 510] f32   (stride 1, VALID padding, + bias)

Data-parallel over batch: 2 images per core across 8 NeuronCores.

v5 design — bf16 HBM I/O + host-side layout swizzle (rel-err budget 2e-2):
 - host pre-swizzles x into the exact SBUF strip layout, bf16: every input
   DMA descriptor is 4KB contiguous; two 0.5MB dma_starts per strip on
   different queues (halves strip-arrival latency, denser engine usage).
   xs[img*16+s, 32g+ci, 512k+w] = x[img, ci, 32s+4k+g, w]
 - mod-4 row rotation: partition group g holds rows r = g (mod 4).  An
   output row's 3 kh taps land in 3 cyclically-consecutive groups, so each
   row needs exactly one K=64 matmul on PE row-half T0 (parts 0-63) and one
   on T1 (64-127) per kw: 48 MMs per 8-row block (75% PE efficiency),
   uniform 3-MM accumulation groups, no zero padding.  Banks stay
   tile-pure (T0 -> psA, T1 -> psB): cross-row-tile accumulation into one
   bank fails on HW (verified).
 - weights: host builds the 12 [64,32] bf16 variants directly; one DMA.
 - drain: ACT Identity(psB + bias) -> t (f32); DVE t + psA -> ostrip bf16.
 - output bf16 in drain-native layout (8160B descriptors, one 1MB
   dma_start per strip); host de-swizzles and upcasts to f32.
"""
import numpy as np
import ml_dtypes

import concourse.bass as bass
import concourse.tile as tile
from concourse import bacc, mybir
from concourse.bass_utils import run_bass_kernel_spmd
from contextlib import ExitStack

F32 = mybir.dt.float32
BF16 = mybir.dt.bfloat16
BF = ml_dtypes.bfloat16

N_FULL, C, H, W = 16, 32, 512, 512
HO = WO = 510
N_CORES = 8
N_PER = N_FULL // N_CORES
N_STRIPS = H // 32
NS = N_PER * N_STRIPS

PATTERNS = [(0, 1), (None, 0), (2, None), (1, 2)]
ROW_TABLE = [(0, 0, 2, 0),
             (1, 0, 3, 0),
             (2, 1, 0, 0),
             (3, 1, 1, 0)]


def _build():
    nc = bacc.Bacc("TRN2", target_bir_lowering=False, debug=False, num_devices=1)
    x_d = nc.dram_tensor("xs", [NS, 128, 4096], BF16, kind="ExternalInput").ap()
    w_d = nc.dram_tensor("wb", [128, 384], BF16, kind="ExternalInput").ap()
    b_d = nc.dram_tensor("bt", [128, 1], F32, kind="ExternalInput").ap()
    o_d = nc.dram_tensor("out", [NS, 128, 4080], BF16, kind="ExternalOutput").ap()

    with tile.TileContext(nc) as tc, ExitStack() as ctx:
        const_pool = ctx.enter_context(tc.tile_pool(name="const", bufs=1))
        xb_pool = ctx.enter_context(tc.tile_pool(name="xb", bufs=6))
        psum_pool = ctx.enter_context(tc.tile_pool(name="ps", bufs=2, space="PSUM"))
        t_pool = ctx.enter_context(tc.tile_pool(name="t", bufs=6))
        out_pool = ctx.enter_context(tc.tile_pool(name="ostrip", bufs=4))

        wb = const_pool.tile([128, 384], BF16)
        nc.sync.dma_start(wb[:], w_d[:])
        bt = const_pool.tile([128, 1], F32)
        nc.scalar.dma_start(bt[:], b_d[:])

        engs = [nc.sync, nc.scalar, nc.gpsimd]
        rr = [0]

        def in_dma(dst, src):
            engs[rr[0] % 3].dma_start(dst, src)
            rr[0] += 1

        def out_dma(dst, src):
            engs[(rr[0] + 1) % 3].dma_start(dst, src)
            rr[0] += 1

        uid = [0]

        def emit_block(xcur, xnext, b8, ostrip, nrow=8):
            """48 MMs + 2 drains for one block (rows y0..y0+nrow-1,
            y0 = 32s + 8*b8)."""
            uid[0] += 1
            psA = {}
            psB = {}
            for sub in range(2):
                psA[sub] = psum_pool.tile([128, 512], F32, tag=f"psA{sub}",
                                          name=f"psA{sub}_{uid[0]}")
                psB[sub] = psum_pool.tile([128, 512], F32, tag=f"psB{sub}",
                                          name=f"psB{sub}_{uid[0]}")
            for kw in range(3):
                for p in range(nrow):
                    j, sub = p // 2, p % 2
                    k = 2 * b8 + p // 4
                    p0v, d0, p1v, d1 = ROW_TABLE[p % 4]
                    for T, pv, dd, ps in ((0, p0v, d0, psA[sub]),
                                          (1, p1v, d1, psB[sub])):
                        sl = k + dd
                        xa = xcur
                        if sl >= 8:
                            xa = xnext
                            sl -= 8
                        off = sl * 512 + kw
                        base = 64 * T
                        v = pv * 3 + kw
                        nc.tensor.matmul(
                            ps[32 * j:32 * j + 32, 0:WO],
                            wb[base:base + 64, 32 * v:32 * v + 32],
                            xa[base:base + 64, off:off + WO],
                            start=(kw == 0), stop=(kw == 2),
                            skip_group_check=True,
                            tile_position=(base, 32 * j),
                        )
            npart = 32 * ((nrow + 1) // 2)
            for sub in range(2):
                uid[0] += 1
                t = t_pool.tile([128, WO], F32, tag="t", name=f"t_{uid[0]}")
                nc.scalar.activation(t[0:npart, :], psB[sub][0:npart, 0:WO],
                                     mybir.ActivationFunctionType.Identity,
                                     bias=bt[0:npart, :])
                nc.vector.tensor_add(
                    ostrip[0:npart, 1020 * b8 + 510 * sub:
                           1020 * b8 + 510 * sub + WO],
                    t[0:npart, :], psA[sub][0:npart, 0:WO])

        for n in range(N_PER):
            prev = None
            for s in range(N_STRIPS):
                uid[0] += 1
                xb = xb_pool.tile([128, 4096], BF16, tag="xb",
                                  name=f"xb_{uid[0]}")
                # two half-strip loads on different queues
                in_dma(xb[:, 0:2048], x_d[n * N_STRIPS + s, :, 0:2048])
                in_dma(xb[:, 2048:4096], x_d[n * N_STRIPS + s, :, 2048:4096])

                if prev is not None:
                    xbp, osp, sp = prev
                    emit_block(xbp, xb, 3, osp)
                    out_dma(o_d[n * N_STRIPS + sp], osp[:])
                uid[0] += 1
                ostrip = out_pool.tile([128, 4080], BF16, tag="ostrip",
                                       name=f"os_{uid[0]}")
                for b8 in range(3):
                    emit_block(xb, None, b8, ostrip)
                if s < N_STRIPS - 1:
                    prev = (xb, ostrip, s)
                else:
                    emit_block(xb, None, 3, ostrip, nrow=6)
                    idx = n * N_STRIPS + s
                    out_dma(o_d[idx, 0:96], ostrip[0:96, :])
                    out_dma(o_d[idx, 96:128, 0:3060], ostrip[96:128, 0:3060])
                    prev = None

    nc.compile()
    return nc


def _prep_x(x):
    """[16, 32, 512, 512] f32 -> per-core list of [32, 128, 4096] bf16."""
    xb = x.astype(BF)
    cores = []
    for c in range(N_CORES):
        imgs = []
        for n in range(N_PER):
            im = xb[c * N_PER + n]                      # [32, 512, 512]
            im = im.reshape(C, N_STRIPS, 8, 4, W)       # ci, s, k, g, w
            im = im.transpose(1, 3, 0, 2, 4)            # s, g, ci, k, w
            imgs.append(np.ascontiguousarray(im.reshape(N_STRIPS, 128, 4096)))
        cores.append(np.concatenate(imgs, axis=0))
    return cores


def _prep_w(weight):
    """[32, 32, 3, 3] f32 -> [128, 384] bf16 variant image."""
    wb = np.zeros((128, 384), dtype=np.float32)
    for pi, (a, b) in enumerate(PATTERNS):
        for kw in range(3):
            col = 32 * (pi * 3 + kw)
            for T in (0, 64):
                if a is not None:
                    wb[T:T + 32, col:col + 32] = weight[:, :, a, kw].T
                if b is not None:
                    wb[T + 32:T + 64, col:col + 32] = weight[:, :, b, kw].T
    return wb.astype(BF)


def _unprep_out(o_arrs):
    """per-core [32, 128, 4080] bf16 -> [16, 32, 510, 510] f32."""
    full = np.empty((N_FULL, C, HO, WO), dtype=np.float32)
    for c, arr in enumerate(o_arrs):
        a = np.asarray(arr).reshape(N_PER, N_STRIPS, 4, 32, 4, 2, WO)
        # dims: n, s, j, co, b8, e, w  ->  n, co, s, b8, j, e, w
        a = a.transpose(0, 3, 1, 4, 2, 5, 6).reshape(N_PER, C, 512, WO)
        full[c * N_PER:(c + 1) * N_PER] = a[:, :, :HO, :].astype(np.float32)
    return full


_NC = None


def prepare_in_maps(x, weight, bias):
    x = np.ascontiguousarray(np.asarray(x, dtype=np.float32))
    weight = np.ascontiguousarray(np.asarray(weight, dtype=np.float32))
    bias = np.ascontiguousarray(np.asarray(bias, dtype=np.float32))
    xs = _prep_x(x)
    wb = _prep_w(weight)
    bt = np.repeat(bias.reshape(1, 32), 4, axis=0).reshape(128, 1)
    bt = np.ascontiguousarray(bt, dtype=np.float32)
    return [{"xs": xs[i], "wb": wb, "bt": bt} for i in range(N_CORES)]


def kernel(x, weight, bias):
    global _NC
    if _NC is None:
        _NC = _build()
    in_maps = prepare_in_maps(x, weight, bias)
    res = run_bass_kernel_spmd(_NC, in_maps, core_ids=list(range(N_CORES)))
    return _unprep_out([r["out"] for r in res.results])


# revision 8
# speedup vs baseline: 2.5875x; 1.3851x over previous
"""Trainium2 Bass kernel for CustomConv2d:
  x [16, 32, 512, 512] f32, weight [32, 32, 3, 3] f32, bias [32] f32
  -> out [16, 32, 510, 510] f32   (stride 1, VALID padding, + bias)

Data-parallel over batch: 2 images per core across 8 NeuronCores.

v6 design — bf16 HBM I/O, host-side swizzle, M=64 tap-pair matmuls:
 - host pre-swizzles x into the SBUF strip layout, bf16 (4KB descriptors,
   two 0.5MB dma_starts per strip):
   xs[img*16+s, 32g+ci, 512k+w] = x[img, ci, 32s+4k+g, w]
 - mod-4 row rotation: partition group g holds rows r = g (mod 4).
 - tap-row pairs: one K=64 M=64 matmul streams an adjacent row pair
   (g0,g1 or g2,g3) once and feeds TWO output rows.  Per out-row quad
   (4 rows, bank-aligned): psA (T0 rows) and psB (T1 rows) each take
   6 MMs (3 kw x 2 col-halves).  12 MMs per quad instead of 24 —
   halves both the matmul count and the LDWEIGHTS occupancy that bound
   v5 (uniform 64x64 tiling, no mode switches).
     psA_q: tile(0,0)=MM_a(pair@q), tile(0,64)=MM_b(pair@q+1)
     psB_q: tile(64,64)=MM_a(pair@q), tile(64,0)=MM_b(pair@q+1)
   pattern A: cols(out r):   g_lo=kh0, g_hi=kh1; cols(out r+1): g_lo=0,  g_hi=kh0
   pattern B: cols(out r-2): g_lo=kh2, g_hi=0;   cols(out r-1): g_lo=kh1, g_hi=kh2
 - drain: ACT Identity(psB + bias) -> t (f32); DVE t + psA -> ostrip bf16.
 - output bf16: ostrip[32p+co, 510*q + w] = out row 32s+4q+p; one 1MB
   dma_start per strip (8160B descriptors); host de-swizzles + upcasts.
"""
import numpy as np
import ml_dtypes

import concourse.bass as bass
import concourse.tile as tile
from concourse import bacc, mybir
from concourse.bass_utils import run_bass_kernel_spmd
from contextlib import ExitStack

F32 = mybir.dt.float32
BF16 = mybir.dt.bfloat16
BF = ml_dtypes.bfloat16

N_FULL, C, H, W = 16, 32, 512, 512
HO = WO = 510
N_CORES = 8
N_PER = N_FULL // N_CORES
N_STRIPS = H // 32
NS = N_PER * N_STRIPS


def _build():
    nc = bacc.Bacc("TRN2", target_bir_lowering=False, debug=False, num_devices=1)
    x_d = nc.dram_tensor("xs", [NS, 128, 4096], BF16, kind="ExternalInput").ap()
    w_d = nc.dram_tensor("wb", [128, 384], BF16, kind="ExternalInput").ap()
    b_d = nc.dram_tensor("bt", [128, 1], F32, kind="ExternalInput").ap()
    o_d = nc.dram_tensor("out", [NS, 128, 4080], BF16, kind="ExternalOutput").ap()

    with tile.TileContext(nc) as tc, ExitStack() as ctx:
        const_pool = ctx.enter_context(tc.tile_pool(name="const", bufs=1))
        xb_pool = ctx.enter_context(tc.tile_pool(name="xb", bufs=6))
        psum_pool = ctx.enter_context(tc.tile_pool(name="ps", bufs=4, space="PSUM"))
        t_pool = ctx.enter_context(tc.tile_pool(name="t", bufs=6))
        out_pool = ctx.enter_context(tc.tile_pool(name="ostrip", bufs=4))

        wb = const_pool.tile([128, 384], BF16)
        nc.sync.dma_start(wb[:], w_d[:])
        bt = const_pool.tile([128, 1], F32)
        nc.scalar.dma_start(bt[:], b_d[:])

        engs = [nc.sync, nc.scalar, nc.gpsimd]
        rr = [0]

        def in_dma(dst, src):
            engs[rr[0] % 3].dma_start(dst, src)
            rr[0] += 1

        def out_dma(dst, src):
            engs[(rr[0] + 1) % 3].dma_start(dst, src)
            rr[0] += 1

        uid = [0]

        # weight column layout: 32*(ab*6 + kw*2 + colhalf) for ab in {A=0,B=1}
        def wcol(ab, kw, ch):
            return 32 * (ab * 6 + kw * 2 + ch)

        def emit_quad(q, xcur, xnext, ostrip, nrow=4):
            """One out-row quad (rows y0..y0+3, y0 = 32s + 4q).
            MM_a from pair@slot q, MM_b from pair@slot q+1 (may be xnext).
            nrow=2 for the final quad of an image (skips MM_b side and
            drains only partitions 0..63)."""
            uid[0] += 1
            psA = psum_pool.tile([128, 512], F32, tag="psA",
                                 name=f"psA_{uid[0]}")
            psB = psum_pool.tile([128, 512], F32, tag="psB",
                                 name=f"psB_{uid[0]}")
            xa_b = xnext if q == 7 else xcur
            sl_b = 0 if q == 7 else q + 1
            for kw in range(3):
                # (bank, row-half T, a/b, col tile, x tile, slot)
                # psA: MM_a = T0 pair of this quad, MM_b = T0 pair of the
                # NEXT quad (rows 4Q+4,4Q+5).  psB: both MMs use this
                # quad's T1 pair (rows 4Q+2,4Q+3).
                mms = [(psA, 0, 0, 0, xcur, q)]
                if nrow == 4:
                    mms += [(psB, 1, 0, 64, xcur, q),
                            (psA, 0, 1, 64, xa_b, sl_b)]
                mms.append((psB, 1, 1, 0, xcur, q))
                for ps, T, ab, ct, xa, sl in mms:
                    base = 64 * T
                    off = sl * 512 + kw
                    nc.tensor.matmul(
                        ps[ct:ct + 64, 0:WO],
                        wb[base:base + 64, wcol(ab, kw, 0):wcol(ab, kw, 0) + 64],
                        xa[base:base + 64, off:off + WO],
                        start=(kw == 0), stop=(kw == 2),
                        skip_group_check=True,
                        tile_position=(base, ct),
                    )
            npart = 32 * nrow
            uid[0] += 1
            t = t_pool.tile([128, WO], F32, tag="t", name=f"t_{uid[0]}")
            nc.scalar.activation(t[0:npart, :], psB[0:npart, 0:WO],
                                 mybir.ActivationFunctionType.Identity,
                                 bias=bt[0:npart, :])
            nc.vector.tensor_add(
                ostrip[0:npart, 510 * q:510 * q + WO],
                t[0:npart, :], psA[0:npart, 0:WO])

        for n in range(N_PER):
            xb = {}
            ost = {}
            for s in range(N_STRIPS):
                uid[0] += 1
                xb[s] = xb_pool.tile([128, 4096], BF16, tag="xb",
                                     name=f"xb_{uid[0]}")
                in_dma(xb[s][:, 0:2048], x_d[n * N_STRIPS + s, :, 0:2048])
                in_dma(xb[s][:, 2048:4096], x_d[n * N_STRIPS + s, :, 2048:4096])
                uid[0] += 1
                ost[s] = out_pool.tile([128, 4080], BF16, tag="ostrip",
                                       name=f"os_{uid[0]}")
                if s >= 1:
                    for q in range(8):
                        emit_quad(q, xb[s - 1], xb[s] if q == 7 else None,
                                  ost[s - 1])
                    out_dma(o_d[n * N_STRIPS + s - 1], ost[s - 1][:])
            s = N_STRIPS - 1
            for q in range(8):
                emit_quad(q, xb[s], None, ost[s], nrow=4 if q < 7 else 2)
            idx = n * N_STRIPS + s
            out_dma(o_d[idx, 0:64], ost[s][0:64, :])
            out_dma(o_d[idx, 64:128, 0:3570], ost[s][64:128, 0:3570])

    nc.compile()
    return nc


def _prep_x(x):
    """[16, 32, 512, 512] f32 -> per-core list of [32, 128, 4096] bf16."""
    xb = x.astype(BF)
    cores = []
    for c in range(N_CORES):
        imgs = []
        for n in range(N_PER):
            im = xb[c * N_PER + n]                      # [32, 512, 512]
            im = im.reshape(C, N_STRIPS, 8, 4, W)       # ci, s, k, g, w
            im = im.transpose(1, 3, 0, 2, 4)            # s, g, ci, k, w
            imgs.append(np.ascontiguousarray(im.reshape(N_STRIPS, 128, 4096)))
        cores.append(np.concatenate(imgs, axis=0))
    return cores


def _prep_w(weight):
    """[32, 32, 3, 3] f32 -> [128, 384] bf16.
    Column layout: 32*(ab*6 + kw*2 + colhalf); both K-halves (partitions
    0-63 and 64-127) carry the same content.
      A: [[kh0, 0], [kh1, kh0]]   (K-half x col-half)
      B: [[kh2, kh1], [0, kh2]]
    """
    wb = np.zeros((128, 384), dtype=np.float32)
    wt = {kh: weight[:, :, kh, :] for kh in range(3)}
    for kw in range(3):
        for T in (0, 64):
            for ab, pat in ((0, ((0, None), (1, 0))), (1, ((2, 1), (None, 2)))):
                c0 = 32 * (ab * 6 + kw * 2)
                for gl in range(2):          # K sub-half (g_lo, g_hi)
                    for ch in range(2):      # col half (out row 0/1 of pair)
                        kh = pat[gl][ch]
                        if kh is not None:
                            wb[T + 32 * gl:T + 32 * gl + 32,
                               c0 + 32 * ch:c0 + 32 * ch + 32] = \
                                weight[:, :, kh, kw].T
    return wb.astype(BF)


def _unprep_out(o_arrs):
    """per-core [32, 128, 4080] bf16 -> [16, 32, 510, 510] f32."""
    full = np.empty((N_FULL, C, HO, WO), dtype=np.float32)
    for c, arr in enumerate(o_arrs):
        a = np.asarray(arr).reshape(N_PER, N_STRIPS, 4, 32, 8, WO)
        # dims: n, s, p, co, q, w  ->  n, co, s, q, p, w
        a = a.transpose(0, 3, 1, 4, 2, 5).reshape(N_PER, C, 512, WO)
        full[c * N_PER:(c + 1) * N_PER] = a[:, :, :HO, :].astype(np.float32)
    return full


_NC = None


def prepare_in_maps(x, weight, bias):
    x = np.ascontiguousarray(np.asarray(x, dtype=np.float32))
    weight = np.ascontiguousarray(np.asarray(weight, dtype=np.float32))
    bias = np.ascontiguousarray(np.asarray(bias, dtype=np.float32))
    xs = _prep_x(x)
    wb = _prep_w(weight)
    bt = np.repeat(bias.reshape(1, 32), 4, axis=0).reshape(128, 1)
    bt = np.ascontiguousarray(bt, dtype=np.float32)
    return [{"xs": xs[i], "wb": wb, "bt": bt} for i in range(N_CORES)]


def kernel(x, weight, bias):
    global _NC
    if _NC is None:
        _NC = _build()
    in_maps = prepare_in_maps(x, weight, bias)
    res = run_bass_kernel_spmd(_NC, in_maps, core_ids=list(range(N_CORES)))
    return _unprep_out([r["out"] for r in res.results])
